# revision 22
# baseline (speedup 1.0000x reference)
"""Multi-head causal attention (B=4, S=2048, H=16, d=64, EMB=1024) on 8 trn2 cores.

Sharding: core c handles batch b = c // 2 and head-group g = c % 2
(8 of 16 heads), i.e. a 512-wide slice of the QKV projection dims.

Device kernel (per core), all matmul inputs fp16 (fp32 PSUM accumulation):
  - Q^T, K^T projections in [dims, tokens] layout (contraction EMB on
    partitions; x is transposed on host), V in [tokens, dims] layout with
    a ones-column appended per head (denominator trick).
  - Scores computed transposed: S^T[kv, q] = lhsT(K^T) .T @ rhs(Q^T), so
    softmax'd probabilities feed the PV matmul directly as rhs with
    lhsT = [V | 1]: Z'[65, q], row 64 = softmax denominator.
  - exp: ScalarE activation PSUM->SBUF, no max-subtraction (|scores| < ~3).
  - Causal diagonal blocks: multiplicative 0/1 mask on DVE after exp.
Host: x/w pre-arranged into the exact SBUF layouts so every DMA is fully
contiguous (128 partition rows x multi-KB runs -> few descriptors, fast
HW DGE triggers); 1/sqrt(d) folded into w_q; final divide-by-denominator
+ head concat + b_v add on host.

Perf notes (vs. naive schedule):
  - dummy matmul stream at kernel start keeps the PE HAM clock gate at
    8/8 (2.4 GHz) through the input-DMA window
  - input DMA triggers split across both HW DGE queues (Sync + Scalar)
  - PSUM: "big" ring bufs=3 (6 banks) + zps bufs=2, PSUM evacuation
    (bias adds, V copies, z copies) on GpSimd so DVE latency never gates
    PE bank reuse
"""

import os
import sys

import numpy as np

for _p in ("/opt/trn_rl_repo",):
    if _p not in sys.path:
        sys.path.insert(0, _p)

import concourse.bass as bass
import concourse.bacc as bacc
import concourse.mybir as mybir
from concourse.tile import TileContext
from concourse.bass_utils import run_bass_kernel_spmd

EMB, QK, V, H = 1024, 64, 64, 16
B, S = 4, 2048
NCORE = 8
HPC = H // 2            # heads per core
DPC = HPC * QK          # projection dims per core (512)
VW = V + 1              # V plus ones-column (65)
NE = EMB // 128         # 8 contraction blocks
ND = DPC // 128         # 4 dim blocks
NQ = S // 512           # 4 q tiles
NT = S // 128           # 16 kv/token blocks
F32 = mybir.dt.float32
F16 = mybir.dt.float16
EXP = mybir.ActivationFunctionType.Exp
XW = NE * 512           # stripe width in sbuf columns (4096)

_cache = {}
last_results = None


def _build_nc():
    nc = bacc.Bacc(None, target_bir_lowering=False)
    # all inputs pre-arranged on host into exact SBUF layouts (contiguous DMA)
    x_qT = nc.declare_dram_parameter("x_qT", [128, NQ * XW], F16, isOutput=False)
    x_kT = nc.declare_dram_parameter("x_kT", [128, NQ * XW], F16, isOutput=False)
    w_qT = nc.declare_dram_parameter("w_qT", [128, NE * DPC], F16, isOutput=False)
    w_kT = nc.declare_dram_parameter("w_kT", [128, NE * DPC], F16, isOutput=False)
    w_vT = nc.declare_dram_parameter("w_vT", [128, NE * DPC], F16, isOutput=False)
    b_qk = nc.declare_dram_parameter("b_qk", [128, 2 * ND], F32, isOutput=False)
    consts = nc.declare_dram_parameter("consts", [128, 4 * 512], F16, isOutput=False)
    z_raw = nc.declare_dram_parameter("z_raw", [HPC, VW, S], F16, isOutput=True)

    with TileContext(nc) as tc:
        with tc.tile_pool(name="const", bufs=1) as cp, \
             tc.tile_pool(name="xin", bufs=8) as xp, \
             tc.tile_pool(name="pt", bufs=6) as pp, \
             tc.tile_pool(name="zout", bufs=2 * HPC) as zo:
            # persistent SBUF tensors
            wq_sb = cp.tile([128, NE * DPC], F16)
            wk_sb = cp.tile([128, NE * DPC], F16)
            wv_sb = cp.tile([128, NE * DPC], F16)
            bqk_sb = cp.tile([128, 2 * ND], F32)
            cm_sb = cp.tile([128, 4 * 512], F16)
            QT = cp.tile([128, ND * S], F16)     # [dim-in-dblk, dblk*S + tok]
            KT = cp.tile([128, ND * S], F16)
            VP = cp.tile([128, NT * HPC * VW], F16)  # [tok-in-blk, blk*520 + h*65 + d]

            # warm-up scratch: dummy matmuls with no data deps keep the PE
            # HAM clock gate at 8/8 (2.4 GHz) through the input-DMA window
            warm = cp.tile([128, 512], F16)
            nc.vector.memset(warm[:, :], 0.5)

            bq_sb, bk_sb = bqk_sb[:, 0:ND], bqk_sb[:, ND:2 * ND]
            um_sb = cm_sb[:, 0:4 * 512]
            # ones columns for the denominator trick (V copies leave col 64)
            nc.vector.memset(VP[:, :], 1.0)
            # pre-warm DVE's vector clock so later DVE ops don't each carry
            # DMA-sem waits (walrus wait-slot limits)
            scr = cp.tile([128, 2], F32)
            scrh = cp.tile([128, 1], F16)
            nc.vector.tensor_copy(scr[:, 0:1], bqk_sb[:, 0:1])
            nc.vector.tensor_copy(scrh[:, 0:1], cm_sb[:, 0:1])
            # pre-warm PE's clock too (dummy weight loads): fused LW+MM pairs
            # have a ~2-slot combined sync-wait budget in walrus codegen, so
            # absorb the const-DMA and DVE deps before real matmuls start
            for ap in (wq_sb, wk_sb, wv_sb, cm_sb, scrh):
                nc.tensor.ldweights(ap[0:64, 0:1])

            # dummy matmul stream covering the gap until the first real
            # matmul's inputs land: keeps HAM at 2.4 GHz, costs nothing
            # (PE would idle anyway), outputs discarded
            with tc.tile_pool(name="wmp", bufs=1, space="PSUM") as wp:
                wps = wp.tile([128, 512], F32)
                for _ in range(20):
                    nc.tensor.matmul(wps[:, :], lhsT=warm[:, 0:128],
                                     rhs=warm[:, :], start=True, stop=True,
                                     skip_group_check=True)

            # ---- input DMAs: first-use order, split across both HW DGE
            # queues (sync + scalar) for parallel descriptor generation ----
            sxq = [None] * NQ
            sxk = [None] * NQ
            for qb in range(NQ):
                sxk[qb] = xp.tile([128, XW], F16, tag="xtb", name=f"sxk{qb}")
                sxq[qb] = xp.tile([128, XW], F16, tag="xtb", name=f"sxq{qb}")

            HW2 = XW // 2
            HV2 = (NE * DPC) // 2
            # two parallel HW DGE queues (~216 GB/s each); first-use order,
            # earliest tensors split in halves across both queues
            XQ4, WQ4 = XW // 4, (NE * DPC) // 4
            for qt in range(4):   # e-pair chunks, arrival matches MM order
                nc.sync.dma_start(out=sxk[0][:, qt * XQ4:(qt + 1) * XQ4],
                                  in_=x_kT[:, qt * XQ4:(qt + 1) * XQ4])
                nc.scalar.dma_start(out=wv_sb[:, qt * WQ4:(qt + 1) * WQ4],
                                    in_=w_vT[:, qt * WQ4:(qt + 1) * WQ4])
            nc.sync.dma_start(out=wq_sb[:, :], in_=w_qT[:, :])
            nc.scalar.dma_start(out=wk_sb[:, :], in_=w_kT[:, :])
            nc.sync.dma_start(out=bqk_sb[:, :], in_=b_qk[:, :])
            nc.sync.dma_start(out=sxq[0][:, 0:HW2], in_=x_qT[:, 0:HW2])
            nc.scalar.dma_start(out=sxq[0][:, HW2:XW], in_=x_qT[:, HW2:XW])
            nc.scalar.dma_start(out=cm_sb[:, :], in_=consts[:, :])
            nc.scalar.dma_start(out=sxk[1][:, :], in_=x_kT[:, XW:2 * XW])
            nc.sync.dma_start(out=sxq[1][:, :], in_=x_qT[:, XW:2 * XW])
            for qb in range(2, NQ):
                nc.sync.dma_start(out=sxk[qb][:, :],
                                  in_=x_kT[:, qb * XW:(qb + 1) * XW])
                nc.scalar.dma_start(out=sxq[qb][:, :],
                                    in_=x_qT[:, qb * XW:(qb + 1) * XW])

            with tc.tile_pool(name="pj", bufs=2, space="PSUM") as pj:
                # V[t, d] with ones column; must finish before attention
                def proj_v(tb):
                    qb, t = divmod(tb, 4)
                    ps = pj.tile([128, 512], F32, tag="big", bufs=3, name=f"pv{tb}")
                    for e in range(NE):
                        nc.tensor.matmul(
                            ps[:, :],
                            lhsT=sxk[qb][:, e * 512 + t * 128: e * 512 + (t + 1) * 128],
                            rhs=wv_sb[:, e * DPC:(e + 1) * DPC],
                            start=(e == 0), stop=(e == NE - 1))
                    dst = VP[:, tb * (HPC * VW):(tb + 1) * (HPC * VW)]
                    dst = dst.rearrange("p (h w) -> p h w", w=VW)[:, :, 0:V]
                    nc.vector.tensor_copy(
                        dst, ps[:, :].rearrange("p (h w) -> p h w", w=V))

                # K^T / Q^T chunk for one (dblk, qb)
                def proj_kq(which, dblk, qb):
                    wsb, bsb, OUT, sx = ((wk_sb, bk_sb, KT, sxk) if which == "k"
                                         else (wq_sb, bq_sb, QT, sxq))
                    ps = pj.tile([128, 512], F32, tag="big", bufs=3,
                                 name=f"p{which}{dblk}{qb}")
                    for e in range(NE):
                        nc.tensor.matmul(
                            ps[:, :],
                            lhsT=wsb[:, e * DPC + dblk * 128: e * DPC + (dblk + 1) * 128],
                            rhs=sx[qb][:, e * 512:(e + 1) * 512],
                            start=(e == 0), stop=(e == NE - 1))
                    nc.vector.tensor_scalar_add(
                        OUT[:, dblk * S + qb * 512: dblk * S + (qb + 1) * 512],
                        ps[:, :], bsb[:, dblk:dblk + 1])

                # prologue: only what (dblk 0, jq 0) needs — the rest is
                # fed into the attention stream in dependency order
                for tb in range(4):
                    proj_v(tb)
                proj_kq("k", 0, 0)
                proj_kq("q", 0, 0)

                # attention for head pair (2*dblk, 2*dblk+1): the two heads'
                # matmuls are interleaved (alternating PE row-groups, so
                # LDWEIGHTS pulls ahead) and one head's matmuls cover the
                # other's exp latency; proj chunks keep PE dense
                def attention_pair(dblk, feed_by_jq):
                    heads = (2 * dblk, 2 * dblk + 1)
                    poffs = (0, 64)

                    def emit_pv(g, pts, zps, nkv):
                        for bs in range(2):
                            for hi in (0, 1):
                                i = 2 * g + bs
                                nc.tensor.matmul(
                                    zps[hi][:, :],
                                    lhsT=VP[:, i * (HPC * VW) + heads[hi] * VW:
                                            i * (HPC * VW) + (heads[hi] + 1) * VW],
                                    rhs=pts[hi][:, bs * 512:(bs + 1) * 512],
                                    start=(i == 0), stop=(i == nkv - 1),
                                    skip_group_check=True)

                    def emit_z(jq, zps):
                        zsb = zo.tile([VW, 1024], F16, tag="zsb", bufs=6,
                                      name=f"zsb{dblk}_{jq}")
                        for hi in (0, 1):
                            nc.vector.tensor_copy(
                                zsb[:, hi * 512:(hi + 1) * 512], zps[hi][:, :])
                        nc.sync.dma_start(
                            out=z_raw[heads[0]:heads[0] + 2, :,
                                      jq * 512:(jq + 1) * 512]
                            .rearrange("h p t -> p h t"),
                            in_=zsb.rearrange("p (h t) -> p h t", h=2))

                    # software pipeline: PV(g-1) is emitted AFTER the scores
                    # of g (also across jq/dblk boundaries), so exp/mask
                    # latency and the z copy-out hide under score streaming
                    for jq in range(NQ):
                            nkv = 4 * (jq + 1)
                            feed = feed_by_jq[jq]
                            qs = slice(dblk * S + jq * 512, dblk * S + (jq + 1) * 512)
                            zps = [pj.tile([VW, 512], F32, tag="zps", bufs=2,
                                           name=f"z{h}_{jq}") for h in heads]
                            for g in range(nkv // 2):
                                for _ in range(2):
                                    if feed:
                                        feed.pop(0)()
                                sps = [pj.tile([128, 1024], F32, tag="big", bufs=3,
                                               name=f"s{hi}") for hi in (0, 1)]
                                for bs in range(2):
                                    for hi in (0, 1):
                                        i = 2 * g + bs
                                        nc.tensor.matmul(
                                            sps[hi][:, bs * 512:(bs + 1) * 512],
                                            lhsT=KT[poffs[hi]:poffs[hi] + 64,
                                                    dblk * S + i * 128:
                                                    dblk * S + (i + 1) * 128],
                                            rhs=QT[poffs[hi]:poffs[hi] + 64, qs],
                                            start=True, stop=True)
                                # diagonal g-iters: columns left of the
                                # 128-wide partial block are fully masked, so
                                # skip them in exp (ScalarE) and zero them on
                                # DVE instead; the 0/1 mask-multiply is only
                                # needed on the 128-col partial block itself
                                bb2 = (2 * g == 4 * jq + 2)  # bb pair (2,3)
                                pts = []
                                for hi in (0, 1):
                                    pt = pp.tile([128, 1024], F16, tag="pt",
                                                 name=f"pt{hi}")
                                    if bb2:
                                        nc.vector.memset(pt[:, 0:256], 0.0)
                                        nc.vector.memset(pt[:, 512:896], 0.0)
                                        nc.scalar.activation(
                                            pt[:, 256:512], sps[hi][:, 256:512], EXP)
                                        nc.scalar.activation(
                                            pt[:, 896:1024], sps[hi][:, 896:1024], EXP)
                                    else:
                                        nc.scalar.activation(pt[:, :], sps[hi][:, :], EXP)
                                    pts.append(pt)
                                for bs in range(2):
                                    i = 2 * g + bs
                                    if i >= 4 * jq:      # diagonal partial block
                                        bb = i - 4 * jq
                                        if bb2:
                                            # fully-masked prefix already
                                            # zeroed via memset: mask only the
                                            # 128-col partial block
                                            lo, w = bs * 512 + 128 * bb, 128
                                            ulo = bb * 512 + 128 * bb
                                        else:
                                            # cover prefix + partial block
                                            lo, w = bs * 512, 128 * (bb + 1)
                                            ulo = bb * 512
                                        for hi in (0, 1):
                                            nc.vector.tensor_mul(
                                                pts[hi][:, lo:lo + w],
                                                pts[hi][:, lo:lo + w],
                                                um_sb[:, ulo:ulo + w])
                                if pend_box[0] is not None:
                                    pend_box[0]()
                                if g < nkv // 2 - 1:
                                    pend_box[0] = (
                                        lambda g=g, pts=pts, zps=zps, nkv=nkv:
                                        emit_pv(g, pts, zps, nkv))
                                else:
                                    pend_box[0] = (
                                        lambda g=g, pts=pts, zps=zps, nkv=nkv,
                                        jq=jq: (emit_pv(g, pts, zps, nkv),
                                                emit_z(jq, zps)))
                            for f in feed:
                                f()
                            feed.clear()

                # proj chunks paced so every jq iteration carries ~2 of
                # them: the PE then always has more queued work per g-iter
                # than ScalarE's exp stream, so exp latency never gates PSUM
                # bank reuse (feedless stretches are ScalarE-bound otherwise)
                def K(d, q): return lambda: proj_kq("k", d, q)
                def Q(d, q): return lambda: proj_kq("q", d, q)
                def PV(tb): return lambda: proj_v(tb)

                feeds = {
                    0: [[K(1, 0), Q(1, 0), K(0, 1), Q(0, 1)],
                        [PV(4), PV(5), PV(6), PV(7)],
                        [K(0, 2), Q(0, 2), PV(8), PV(9), PV(10), PV(11)],
                        [K(0, 3), Q(0, 3), PV(12), PV(13), PV(14), PV(15)]],
                    1: [[K(1, 1), Q(1, 1)], [K(1, 2), Q(1, 2)],
                        [K(1, 3), Q(1, 3)], [K(2, 0), Q(2, 0)]],
                    2: [[K(2, 1), Q(2, 1)], [K(2, 2), Q(2, 2)],
                        [K(2, 3), Q(2, 3)], [K(3, 0), Q(3, 0)]],
                    3: [[K(3, 1), Q(3, 1)], [K(3, 2), Q(3, 2)],
                        [K(3, 3), Q(3, 3)], []],
                }
                pend_box = [None]
                for dblk in range(ND):
                    attention_pair(dblk, feeds[dblk])
                pend_box[0]()

    nc.compile()
    return nc


def kernel(x_q, x_k_v, attn_mask, w_q, b_q, w_k, b_k, w_v, b_v):
    global last_results
    x_q = np.ascontiguousarray(x_q, np.float32)
    x_k_v = np.ascontiguousarray(x_k_v, np.float32)
    w_q, w_k, w_v = (np.asarray(a, np.float32) for a in (w_q, w_k, w_v))
    b_q, b_k, b_v = (np.asarray(a, np.float32) for a in (b_q, b_k, b_v))

    if "nc" not in _cache:
        _cache["nc"] = _build_nc()
    nc = _cache["nc"]

    scale = 1.0 / np.sqrt(np.float32(QK))

    def x_sb(x):  # [S, EMB] -> [128, NQ*NE*512] sbuf stripe layout
        # sb[p, qb, e, t] = x[qb*512+t, e*128+p]
        a = x.T.reshape(NE, 128, NQ, 512).transpose(1, 2, 0, 3)
        return np.ascontiguousarray(a.reshape(128, NQ * XW)).astype(np.float16)

    def w_sb(w):  # [DPC, EMB] -> [128, NE*DPC]: sb[p, e, d] = w[d, e*128+p]
        a = w.T.reshape(NE, 128, DPC).transpose(1, 0, 2)
        return np.ascontiguousarray(a.reshape(128, NE * DPC)).astype(np.float16)

    xqT = [x_sb(x_q[b]) for b in range(B)]
    xkT = [x_sb(x_k_v[b]) for b in range(B)]
    wqT = [w_sb(w_q[g * DPC:(g + 1) * DPC] * scale) for g in range(2)]
    wkT = [w_sb(w_k[g * DPC:(g + 1) * DPC]) for g in range(2)]
    wvT = [w_sb(w_v[g * DPC:(g + 1) * DPC]) for g in range(2)]
    bq2 = [np.ascontiguousarray((b_q[g * DPC:(g + 1) * DPC] * scale).reshape(ND, 128).T)
           for g in range(2)]
    bk2 = [np.ascontiguousarray(b_k[g * DPC:(g + 1) * DPC].reshape(ND, 128).T)
           for g in range(2)]
    # multiplicative causal masks for the 4 diagonal 128x512 blocks: block bb
    # masks column qq (of 512) on partition p (kv within block) when
    # 128*bb + p > qq
    p = np.arange(128)[:, None]
    qq = np.arange(512)[None, :]
    um = np.concatenate(
        [np.where(128 * bb + p > qq, np.float32(0.0), np.float32(1.0))
         for bb in range(4)], axis=1).astype(np.float32)
    cm = np.ascontiguousarray(um).astype(np.float16)
    bqk2 = [np.ascontiguousarray(np.concatenate([bq2[g], bk2[g]], axis=1))
            for g in range(2)]

    in_maps = []
    for c in range(NCORE):
        b, g = c // 2, c % 2
        in_maps.append({
            "x_qT": xqT[b], "x_kT": xkT[b],
            "w_qT": wqT[g], "w_kT": wkT[g], "w_vT": wvT[g],
            "b_qk": bqk2[g], "consts": cm,
        })

    trace = os.environ.get("KERNEL_TRACE", "") == "1"
    res = run_bass_kernel_spmd(nc, in_maps, list(range(NCORE)), trace=trace)
    last_results = res

    out = np.empty((B, S, H * V), np.float32)
    for c in range(NCORE):
        b, g = c // 2, c % 2
        zr = res.results[c]["z_raw"].astype(np.float32)   # [HPC, VW, S]
        z = zr[:, :V, :] / zr[:, V:VW, :]                  # [HPC, V, S]
        out[b, :, g * DPC:(g + 1) * DPC] = z.transpose(2, 0, 1).reshape(S, DPC)
    out += b_v[None, None, :]
    return out


# revision 23
# speedup vs baseline: 1.0341x; 1.0341x over previous
"""Multi-head causal attention (B=4, S=2048, H=16, d=64, EMB=1024) on 8 trn2 cores.

Sharding: core c handles batch b = c // 2 and head-group g = c % 2
(8 of 16 heads), i.e. a 512-wide slice of the QKV projection dims.

Device kernel (per core), all matmul inputs fp16 (fp32 PSUM accumulation):
  - Q^T, K^T projections in [dims, tokens] layout (contraction EMB on
    partitions; x is transposed on host), V in [tokens, dims] layout with
    a ones-column appended per head (denominator trick).
  - Scores computed transposed: S^T[kv, q] = lhsT(K^T) .T @ rhs(Q^T), so
    softmax'd probabilities feed the PV matmul directly as rhs with
    lhsT = [V | 1]: Z'[65, q], row 64 = softmax denominator.
  - exp: ScalarE activation PSUM->SBUF, no max-subtraction (|scores| < ~3).
  - Causal diagonal blocks: multiplicative 0/1 mask on DVE after exp.
Host: x/w pre-arranged into the exact SBUF layouts so every DMA is fully
contiguous (128 partition rows x multi-KB runs -> few descriptors, fast
HW DGE triggers); 1/sqrt(d) folded into w_q; final divide-by-denominator
+ head concat + b_v add on host.

Perf notes (vs. naive schedule):
  - dummy matmul stream at kernel start keeps the PE HAM clock gate at
    8/8 (2.4 GHz) through the input-DMA window
  - input DMA triggers split across both HW DGE queues (Sync + Scalar)
  - PSUM: "big" ring bufs=3 (6 banks) + zps bufs=2, PSUM evacuation
    (bias adds, V copies, z copies) on GpSimd so DVE latency never gates
    PE bank reuse
"""

import os
import sys

import numpy as np

for _p in ("/opt/trn_rl_repo",):
    if _p not in sys.path:
        sys.path.insert(0, _p)

import concourse.bass as bass
import concourse.bacc as bacc
import concourse.mybir as mybir
from concourse.tile import TileContext
from concourse.bass_utils import run_bass_kernel_spmd

EMB, QK, V, H = 1024, 64, 64, 16
B, S = 4, 2048
NCORE = 8
HPC = H // 2            # heads per core
DPC = HPC * QK          # projection dims per core (512)
VW = V + 1              # V plus ones-column (65)
NE = EMB // 128         # 8 contraction blocks
ND = DPC // 128         # 4 dim blocks
NQ = S // 512           # 4 q tiles
NT = S // 128           # 16 kv/token blocks
F32 = mybir.dt.float32
F16 = mybir.dt.float16
EXP = mybir.ActivationFunctionType.Exp
XW = NE * 512           # stripe width in sbuf columns (4096)

_cache = {}
last_results = None


def _build_nc():
    nc = bacc.Bacc(None, target_bir_lowering=False)
    # all inputs pre-arranged on host into exact SBUF layouts (contiguous DMA)
    x_qT = nc.declare_dram_parameter("x_qT", [128, NQ * XW], F16, isOutput=False)
    x_kT = nc.declare_dram_parameter("x_kT", [128, NQ * XW], F16, isOutput=False)
    w_qT = nc.declare_dram_parameter("w_qT", [128, NE * DPC], F16, isOutput=False)
    w_kT = nc.declare_dram_parameter("w_kT", [128, NE * DPC], F16, isOutput=False)
    w_vT = nc.declare_dram_parameter("w_vT", [128, NE * DPC], F16, isOutput=False)
    b_qk = nc.declare_dram_parameter("b_qk", [128, 2 * ND], F32, isOutput=False)
    consts = nc.declare_dram_parameter("consts", [128, 4 * 512], F16, isOutput=False)
    z_raw = nc.declare_dram_parameter("z_raw", [HPC, VW, S], F16, isOutput=True)

    with TileContext(nc) as tc:
        with tc.tile_pool(name="const", bufs=1) as cp, \
             tc.tile_pool(name="xin", bufs=8) as xp, \
             tc.tile_pool(name="pt", bufs=6) as pp, \
             tc.tile_pool(name="zout", bufs=2 * HPC) as zo:
            # persistent SBUF tensors
            wq_sb = cp.tile([128, NE * DPC], F16)
            wk_sb = cp.tile([128, NE * DPC], F16)
            wv_sb = cp.tile([128, NE * DPC], F16)
            bqk_sb = cp.tile([128, 2 * ND], F32)
            cm_sb = cp.tile([128, 4 * 512], F16)
            QT = cp.tile([128, ND * S], F16)     # [dim-in-dblk, dblk*S + tok]
            KT = cp.tile([128, ND * S], F16)
            VP = cp.tile([128, NT * HPC * VW], F16)  # [tok-in-blk, blk*520 + h*65 + d]

            # warm-up scratch: dummy matmuls with no data deps keep the PE
            # HAM clock gate at 8/8 (2.4 GHz) through the input-DMA window
            warm = cp.tile([128, 512], F16)
            nc.vector.memset(warm[:, :], 0.5)

            bq_sb, bk_sb = bqk_sb[:, 0:ND], bqk_sb[:, ND:2 * ND]
            um_sb = cm_sb[:, 0:4 * 512]
            # ones columns for the denominator trick (V copies leave col 64)
            nc.vector.memset(VP[:, :], 1.0)
            # pre-warm DVE's vector clock so later DVE ops don't each carry
            # DMA-sem waits (walrus wait-slot limits)
            scr = cp.tile([128, 2], F32)
            scrh = cp.tile([128, 1], F16)
            nc.vector.tensor_copy(scr[:, 0:1], bqk_sb[:, 0:1])
            nc.vector.tensor_copy(scrh[:, 0:1], cm_sb[:, 0:1])
            # pre-warm PE's clock too (dummy weight loads): fused LW+MM pairs
            # have a ~2-slot combined sync-wait budget in walrus codegen, so
            # absorb the const-DMA and DVE deps before real matmuls start
            for ap in (wq_sb, wk_sb, wv_sb, cm_sb, scrh):
                nc.tensor.ldweights(ap[0:64, 0:1])

            # dummy matmul stream covering the gap until the first real
            # matmul's inputs land: keeps HAM at 2.4 GHz, costs nothing
            # (PE would idle anyway), outputs discarded
            with tc.tile_pool(name="wmp", bufs=1, space="PSUM") as wp:
                wps = wp.tile([128, 512], F32)
                for _ in range(20):
                    nc.tensor.matmul(wps[:, :], lhsT=warm[:, 0:128],
                                     rhs=warm[:, :], start=True, stop=True,
                                     skip_group_check=True)

            # ---- input DMAs: first-use order, split across both HW DGE
            # queues (sync + scalar) for parallel descriptor generation ----
            sxq = [None] * NQ
            sxk = [None] * NQ
            for qb in range(NQ):
                sxk[qb] = xp.tile([128, XW], F16, tag="xtb", name=f"sxk{qb}")
                sxq[qb] = xp.tile([128, XW], F16, tag="xtb", name=f"sxq{qb}")

            HW2 = XW // 2
            HV2 = (NE * DPC) // 2
            # two parallel HW DGE queues (~216 GB/s each); first-use order,
            # earliest tensors split in halves across both queues
            nc.sync.dma_start(out=sxk[0][:, 0:HW2], in_=x_kT[:, 0:HW2])
            nc.scalar.dma_start(out=wv_sb[:, 0:HV2], in_=w_vT[:, 0:HV2])
            nc.sync.dma_start(out=wv_sb[:, HV2:], in_=w_vT[:, HV2:])
            nc.scalar.dma_start(out=sxk[0][:, HW2:XW], in_=x_kT[:, HW2:XW])
            nc.sync.dma_start(out=wq_sb[:, :], in_=w_qT[:, :])
            nc.scalar.dma_start(out=wk_sb[:, :], in_=w_kT[:, :])
            nc.sync.dma_start(out=bqk_sb[:, :], in_=b_qk[:, :])
            nc.sync.dma_start(out=sxq[0][:, 0:HW2], in_=x_qT[:, 0:HW2])
            nc.scalar.dma_start(out=sxq[0][:, HW2:XW], in_=x_qT[:, HW2:XW])
            nc.scalar.dma_start(out=cm_sb[:, :], in_=consts[:, :])
            nc.scalar.dma_start(out=sxk[1][:, :], in_=x_kT[:, XW:2 * XW])
            nc.sync.dma_start(out=sxq[1][:, :], in_=x_qT[:, XW:2 * XW])
            for qb in range(2, NQ):
                nc.sync.dma_start(out=sxk[qb][:, :],
                                  in_=x_kT[:, qb * XW:(qb + 1) * XW])
                nc.scalar.dma_start(out=sxq[qb][:, :],
                                    in_=x_qT[:, qb * XW:(qb + 1) * XW])

            with tc.tile_pool(name="pj", bufs=2, space="PSUM") as pj:
                # V[t, d] with ones column; must finish before attention
                def proj_v(tb):
                    qb, t = divmod(tb, 4)
                    ps = pj.tile([128, 512], F32, tag="big", bufs=3, name=f"pv{tb}")
                    for e in range(NE):
                        nc.tensor.matmul(
                            ps[:, :],
                            lhsT=sxk[qb][:, e * 512 + t * 128: e * 512 + (t + 1) * 128],
                            rhs=wv_sb[:, e * DPC:(e + 1) * DPC],
                            start=(e == 0), stop=(e == NE - 1))
                    dst = VP[:, tb * (HPC * VW):(tb + 1) * (HPC * VW)]
                    dst = dst.rearrange("p (h w) -> p h w", w=VW)[:, :, 0:V]
                    nc.vector.tensor_copy(
                        dst, ps[:, :].rearrange("p (h w) -> p h w", w=V))

                # K^T / Q^T chunk for one (dblk, qb)
                def proj_kq(which, dblk, qb):
                    wsb, bsb, OUT, sx = ((wk_sb, bk_sb, KT, sxk) if which == "k"
                                         else (wq_sb, bq_sb, QT, sxq))
                    ps = pj.tile([128, 512], F32, tag="big", bufs=3,
                                 name=f"p{which}{dblk}{qb}")
                    for e in range(NE):
                        nc.tensor.matmul(
                            ps[:, :],
                            lhsT=wsb[:, e * DPC + dblk * 128: e * DPC + (dblk + 1) * 128],
                            rhs=sx[qb][:, e * 512:(e + 1) * 512],
                            start=(e == 0), stop=(e == NE - 1))
                    nc.vector.tensor_scalar_add(
                        OUT[:, dblk * S + qb * 512: dblk * S + (qb + 1) * 512],
                        ps[:, :], bsb[:, dblk:dblk + 1])

                # prologue: only what (dblk 0, jq 0) needs — the rest is
                # fed into the attention stream in dependency order
                for tb in range(4):
                    proj_v(tb)
                proj_kq("k", 0, 0)
                proj_kq("q", 0, 0)

                # attention for head pair (2*dblk, 2*dblk+1): the two heads'
                # matmuls are interleaved (alternating PE row-groups, so
                # LDWEIGHTS pulls ahead) and one head's matmuls cover the
                # other's exp latency; proj chunks keep PE dense
                def attention_pair(dblk, feed_by_jq):
                    heads = (2 * dblk, 2 * dblk + 1)
                    poffs = (0, 64)

                    def emit_pv(g, pts, zps, nkv):
                        for bs in range(2):
                            for hi in (0, 1):
                                i = 2 * g + bs
                                nc.tensor.matmul(
                                    zps[hi][:, :],
                                    lhsT=VP[:, i * (HPC * VW) + heads[hi] * VW:
                                            i * (HPC * VW) + (heads[hi] + 1) * VW],
                                    rhs=pts[hi][:, bs * 512:(bs + 1) * 512],
                                    start=(i == 0), stop=(i == nkv - 1),
                                    skip_group_check=True)

                    def emit_z(jq, zps):
                        zsb = zo.tile([VW, 1024], F16, tag="zsb", bufs=6,
                                      name=f"zsb{dblk}_{jq}")
                        for hi in (0, 1):
                            nc.vector.tensor_copy(
                                zsb[:, hi * 512:(hi + 1) * 512], zps[hi][:, :])
                        nc.sync.dma_start(
                            out=z_raw[heads[0]:heads[0] + 2, :,
                                      jq * 512:(jq + 1) * 512]
                            .rearrange("h p t -> p h t"),
                            in_=zsb.rearrange("p (h t) -> p h t", h=2))

                    # software pipeline: PV(g-1) is emitted AFTER the scores
                    # of g (also across jq/dblk boundaries), so exp/mask
                    # latency and the z copy-out hide under score streaming
                    for jq in range(NQ):
                            nkv = 4 * (jq + 1)
                            feed = feed_by_jq[jq]
                            qs = slice(dblk * S + jq * 512, dblk * S + (jq + 1) * 512)
                            zps = [pj.tile([VW, 512], F32, tag="zps", bufs=2,
                                           name=f"z{h}_{jq}") for h in heads]
                            for g in range(nkv // 2):
                                for _ in range(2):
                                    if feed:
                                        feed.pop(0)()
                                sps = [pj.tile([128, 1024], F32, tag="big", bufs=3,
                                               name=f"s{hi}") for hi in (0, 1)]
                                for bs in range(2):
                                    for hi in (0, 1):
                                        i = 2 * g + bs
                                        nc.tensor.matmul(
                                            sps[hi][:, bs * 512:(bs + 1) * 512],
                                            lhsT=KT[poffs[hi]:poffs[hi] + 64,
                                                    dblk * S + i * 128:
                                                    dblk * S + (i + 1) * 128],
                                            rhs=QT[poffs[hi]:poffs[hi] + 64, qs],
                                            start=True, stop=True)
                                # diagonal g-iters: columns left of the
                                # 128-wide partial block are fully masked, so
                                # skip them in exp (ScalarE) and zero them on
                                # DVE instead; the 0/1 mask-multiply is only
                                # needed on the 128-col partial block itself
                                bb2 = (2 * g == 4 * jq + 2)  # bb pair (2,3)
                                pts = []
                                for hi in (0, 1):
                                    pt = pp.tile([128, 1024], F16, tag="pt",
                                                 name=f"pt{hi}")
                                    if bb2:
                                        nc.vector.memset(pt[:, 0:256], 0.0)
                                        nc.vector.memset(pt[:, 512:896], 0.0)
                                        nc.scalar.activation(
                                            pt[:, 256:512], sps[hi][:, 256:512], EXP)
                                        nc.scalar.activation(
                                            pt[:, 896:1024], sps[hi][:, 896:1024], EXP)
                                    else:
                                        nc.scalar.activation(pt[:, :], sps[hi][:, :], EXP)
                                    pts.append(pt)
                                for bs in range(2):
                                    i = 2 * g + bs
                                    if i >= 4 * jq:      # diagonal partial block
                                        bb = i - 4 * jq
                                        if bb2:
                                            # fully-masked prefix already
                                            # zeroed via memset: mask only the
                                            # 128-col partial block
                                            lo, w = bs * 512 + 128 * bb, 128
                                            ulo = bb * 512 + 128 * bb
                                        else:
                                            # cover prefix + partial block
                                            lo, w = bs * 512, 128 * (bb + 1)
                                            ulo = bb * 512
                                        for hi in (0, 1):
                                            nc.vector.tensor_mul(
                                                pts[hi][:, lo:lo + w],
                                                pts[hi][:, lo:lo + w],
                                                um_sb[:, ulo:ulo + w])
                                if pend_box[0] is not None:
                                    pend_box[0]()
                                if g < nkv // 2 - 1:
                                    pend_box[0] = (
                                        lambda g=g, pts=pts, zps=zps, nkv=nkv:
                                        emit_pv(g, pts, zps, nkv))
                                else:
                                    pend_box[0] = (
                                        lambda g=g, pts=pts, zps=zps, nkv=nkv,
                                        jq=jq: (emit_pv(g, pts, zps, nkv),
                                                emit_z(jq, zps)))
                            for f in feed:
                                f()
                            feed.clear()

                # proj chunks paced so every jq iteration carries ~2 of
                # them: the PE then always has more queued work per g-iter
                # than ScalarE's exp stream, so exp latency never gates PSUM
                # bank reuse (feedless stretches are ScalarE-bound otherwise)
                def K(d, q): return lambda: proj_kq("k", d, q)
                def Q(d, q): return lambda: proj_kq("q", d, q)
                def PV(tb): return lambda: proj_v(tb)

                feeds = {
                    0: [[K(1, 0), Q(1, 0), K(0, 1), Q(0, 1)],
                        [PV(4), PV(5), PV(6), PV(7)],
                        [K(0, 2), Q(0, 2), PV(8), PV(9), PV(10), PV(11)],
                        [K(0, 3), Q(0, 3), PV(12), PV(13), PV(14), PV(15)]],
                    1: [[K(1, 1), Q(1, 1)], [K(1, 2), Q(1, 2)],
                        [K(1, 3), Q(1, 3)], [K(2, 0), Q(2, 0)]],
                    2: [[K(2, 1), Q(2, 1)], [K(2, 2), Q(2, 2)],
                        [K(2, 3), Q(2, 3)], [K(3, 0), Q(3, 0)]],
                    3: [[K(3, 1), Q(3, 1)], [K(3, 2), Q(3, 2)],
                        [K(3, 3), Q(3, 3)], []],
                }
                pend_box = [None]
                for dblk in range(ND):
                    attention_pair(dblk, feeds[dblk])
                pend_box[0]()

    nc.compile()
    return nc


def kernel(x_q, x_k_v, attn_mask, w_q, b_q, w_k, b_k, w_v, b_v):
    global last_results
    x_q = np.ascontiguousarray(x_q, np.float32)
    x_k_v = np.ascontiguousarray(x_k_v, np.float32)
    w_q, w_k, w_v = (np.asarray(a, np.float32) for a in (w_q, w_k, w_v))
    b_q, b_k, b_v = (np.asarray(a, np.float32) for a in (b_q, b_k, b_v))

    if "nc" not in _cache:
        _cache["nc"] = _build_nc()
    nc = _cache["nc"]

    scale = 1.0 / np.sqrt(np.float32(QK))

    def x_sb(x):  # [S, EMB] -> [128, NQ*NE*512] sbuf stripe layout
        # sb[p, qb, e, t] = x[qb*512+t, e*128+p]
        a = x.T.reshape(NE, 128, NQ, 512).transpose(1, 2, 0, 3)
        return np.ascontiguousarray(a.reshape(128, NQ * XW)).astype(np.float16)

    def w_sb(w):  # [DPC, EMB] -> [128, NE*DPC]: sb[p, e, d] = w[d, e*128+p]
        a = w.T.reshape(NE, 128, DPC).transpose(1, 0, 2)
        return np.ascontiguousarray(a.reshape(128, NE * DPC)).astype(np.float16)

    xqT = [x_sb(x_q[b]) for b in range(B)]
    xkT = [x_sb(x_k_v[b]) for b in range(B)]
    wqT = [w_sb(w_q[g * DPC:(g + 1) * DPC] * scale) for g in range(2)]
    wkT = [w_sb(w_k[g * DPC:(g + 1) * DPC]) for g in range(2)]
    wvT = [w_sb(w_v[g * DPC:(g + 1) * DPC]) for g in range(2)]
    bq2 = [np.ascontiguousarray((b_q[g * DPC:(g + 1) * DPC] * scale).reshape(ND, 128).T)
           for g in range(2)]
    bk2 = [np.ascontiguousarray(b_k[g * DPC:(g + 1) * DPC].reshape(ND, 128).T)
           for g in range(2)]
    # multiplicative causal masks for the 4 diagonal 128x512 blocks: block bb
    # masks column qq (of 512) on partition p (kv within block) when
    # 128*bb + p > qq
    p = np.arange(128)[:, None]
    qq = np.arange(512)[None, :]
    um = np.concatenate(
        [np.where(128 * bb + p > qq, np.float32(0.0), np.float32(1.0))
         for bb in range(4)], axis=1).astype(np.float32)
    cm = np.ascontiguousarray(um).astype(np.float16)
    bqk2 = [np.ascontiguousarray(np.concatenate([bq2[g], bk2[g]], axis=1))
            for g in range(2)]

    in_maps = []
    for c in range(NCORE):
        b, g = c // 2, c % 2
        in_maps.append({
            "x_qT": xqT[b], "x_kT": xkT[b],
            "w_qT": wqT[g], "w_kT": wkT[g], "w_vT": wvT[g],
            "b_qk": bqk2[g], "consts": cm,
        })

    trace = os.environ.get("KERNEL_TRACE", "") == "1"
    res = run_bass_kernel_spmd(nc, in_maps, list(range(NCORE)), trace=trace)
    last_results = res

    out = np.empty((B, S, H * V), np.float32)
    for c in range(NCORE):
        b, g = c // 2, c % 2
        zr = res.results[c]["z_raw"].astype(np.float32)   # [HPC, VW, S]
        z = zr[:, :V, :] / zr[:, V:VW, :]                  # [HPC, V, S]
        out[b, :, g * DPC:(g + 1) * DPC] = z.transpose(2, 0, 1).reshape(S, DPC)
    out += b_v[None, None, :]
    return out
